# revision 10
# baseline (speedup 1.0000x reference)
"""Trainium2 Bass kernel for convolutional sparse coding (FISTA, 20 iters).

Problem: x (8, 16, 32768) f32, D (32, 16, 64) f32.
    z = FISTA(x, D)  (20 iterations of grad = conv(convt(z, D) - x, D),
    soft-threshold, Nesterov momentum); output = convt(w_final, D).

Strategy (pure data parallel, one batch element per NeuronCore, D replicated):
  * Block-Toeplitz formulation with time blocks of BS=8: both convolutions
    become 9 GEMM "diagonals" of (128-contraction x 128-out x free) matmuls
    at full 128x128 PE utilization.
  * Layouts: z/w live as 2 SBUF tiles with partition (s%4)*32+k (s = in-block
    time, k = atom); y/x/r live as 1 tile with partition u*16+c.  The convt
    output layout IS the conv input layout, so no relayout ever happens and
    the whole 20-iteration FISTA state stays resident in SBUF.
  * Host precomputes the Toeplitz-structured weights from D (plus 1/L and
    the threshold, folded into the conv weights / activation bias), and
    pre-blocks x / un-blocks the output, so the device never transposes.
  * Matmul operands (z/w/r state + weights) are stored bf16 (1 row/cycle on
    the PE, FWL weight loads, DVE 2x modes); gradients accumulate in fp32
    PSUM and the soft-threshold step reads PSUM at full fp32 precision.
    Measured numerics of this scheme: ~2e-3 relative error vs fp32.
"""

import os
import sys

import numpy as np

for _p in (
    "/root/.axon_site",
    "/root/.axon_site/_ro/trn_rl_repo",
    "/root/.axon_site/_ro/pypackages",
    "/opt/trn_rl_repo",
):
    if os.path.isdir(_p) and _p not in sys.path:
        sys.path.append(_p)

import concourse.bass as bass  # noqa: E402
import concourse.tile as tile  # noqa: E402
from concourse import bacc, mybir  # noqa: E402

BS = 8  # time-block size
KS = 64  # conv kernel length
ND = KS // BS + 1  # 9 gemm diagonals
KA = 32  # dictionary atoms
CH = 16  # channels
LMBD = 0.1
N_ITER = 20
FT = 512  # free-dim (block-index) tile size == one PSUM bank of fp32

F32 = mybir.dt.float32


def _lipschitz(D: np.ndarray) -> float:
    F = np.fft.fft(D.astype(np.float64), axis=2)
    L = float(np.sum(np.max((F * np.conj(F)).real, axis=2)))
    return 1.0 if L == 0.0 else L


def _momentum_schedule(n_iter: int) -> list[float]:
    betas = [1.0]
    for _ in range(n_iter):
        b = betas[-1]
        betas.append((1.0 + (1.0 + 4.0 * b * b) ** 0.5) / 2.0)
    return [(betas[i] - 1.0) / betas[i + 1] for i in range(n_iter)]


def make_weights(D: np.ndarray, invL: float) -> np.ndarray:
    """Toeplitz GEMM weights, shape (2*ND*2, 128, 128), fp32.

    index = kind*(2*ND) + d*2 + q
      kind 0 (convt): lhsT[(s%4)*32+k, u*16+c] = D[k, c, 8d+u-s]   (s = 4q+s%4)
      kind 1 (conv):  lhsT[u*16+c, (s%4)*32+k] = -invL*D[k, c, 8d+u-s]
    """
    D = np.asarray(D, np.float32)
    wts = np.zeros((2, ND, 2, 128, 128), np.float32)
    for d in range(ND):
        for q in range(2):
            M = np.zeros((4, KA, BS, CH), np.float32)  # (sl, k, u, c)
            V = np.zeros((BS, CH, 4, KA), np.float32)  # (u, c, sl, k)
            for sl in range(4):
                s = 4 * q + sl
                for u in range(BS):
                    j = BS * d + u - s
                    if 0 <= j < KS:
                        M[sl, :, u, :] = D[:, :, j]  # [k, c]
                        V[u, :, sl, :] = (-invL) * D[:, :, j].T  # [c, k]
            wts[0, d, q] = M.reshape(128, 128)
            wts[1, d, q] = V.reshape(128, 128)
    return wts.reshape(2 * ND * 2, 128, 128)


def block_x(xn: np.ndarray) -> np.ndarray:
    """(C, T) -> (128, T//BS) with partition u*16+c, col b = x[c, 8b+u]."""
    C, T = xn.shape
    return (
        np.ascontiguousarray(
            xn.reshape(C, T // BS, BS).transpose(2, 0, 1)
        ).reshape(128, T // BS)
    )


def unblock_out(ob: np.ndarray, T: int) -> np.ndarray:
    """(128, T//BS) -> (C, T)."""
    return np.ascontiguousarray(
        ob.reshape(BS, CH, T // BS).transpose(1, 2, 0)
    ).reshape(CH, T)


def build_nc(T: int, n_iter: int, mm_dtype: str = "bfloat16", ft: int = FT):
    """Build + compile the single-core FISTA graph (SPMD across 8 cores).

    DRAM params: x (128, NB) f32 [pre-blocked], wts (2*ND*2, 128, 128) f32,
    thrneg (128, 1) f32 [= -lmbd/L], masks (2, 128, 1) f32 [last-z-block
    partition validity], out (128, NB) f32 [blocked output].
    """
    mmdt = getattr(mybir.dt, mm_dtype)  # storage dtype of all matmul operands
    assert T % BS == 0
    NB = T // BS
    Tz = T - KS + 1
    NZB = -(-Tz // BS)
    ZC = NB + BS  # 8 left zero-pad + NZB data + right zero-pad
    RC = NB + 2  # one zero halo block on the right (+1 spare: keep bytes %4)
    y_tiles = [(i * ft, min(ft, NB - i * ft)) for i in range(-(-NB // ft))]
    z_tiles = [(i * ft, min(ft, NZB - i * ft)) for i in range(-(-NZB // ft))]
    moms = _momentum_schedule(n_iter)
    lastz = BS + NZB - 1  # column of the (partially valid) last z block

    nc = bacc.Bacc("TRN2", target_bir_lowering=False, debug=False)
    x_d = nc.declare_dram_parameter("x", [128, NB], F32, isOutput=False)
    wts_d = nc.declare_dram_parameter(
        "wts", [2 * ND * 2, 128, 128], mmdt, isOutput=False
    )
    thr_d = nc.declare_dram_parameter("thrneg", [128, 1], F32, isOutput=False)
    mask_d = nc.declare_dram_parameter("masks", [2, 128, 1], F32, isOutput=False)
    out_d = nc.declare_dram_parameter("out", [128, NB], F32, isOutput=True)

    with tile.TileContext(nc) as tc:
        with (
            tc.tile_pool(name="state", bufs=1) as state,
            tc.tile_pool(name="py", bufs=2, space="PSUM") as py_pool,
            tc.tile_pool(name="pg", bufs=4, space="PSUM") as pg_pool,
            tc.tile_pool(name="tmps", bufs=4) as tmps,
        ):
            X = state.tile([128, NB], F32, name="X")
            R = state.tile([128, RC], mmdt, name="R")
            Z = [state.tile([128, ZC], mmdt, name=f"Zq{q}") for q in range(2)]
            WA = [state.tile([128, ZC], mmdt, name=f"WAq{q}") for q in range(2)]
            WB = [state.tile([128, ZC], mmdt, name=f"WBq{q}") for q in range(2)]
            WT = state.tile([128, 2 * ND * 2 * 128], mmdt, name="WT")
            TH = state.tile([128, 1], F32, name="TH")
            MK = state.tile([128, 2], F32, name="MK")
            OUT = state.tile([128, NB], F32, name="OUTS")

            nc.sync.dma_start(X[:], x_d[:])
            nc.sync.dma_start(
                WT[:].rearrange("p (i j) -> p i j", i=2 * ND * 2),
                wts_d[:].rearrange("i p j -> p i j"),
            )
            nc.sync.dma_start(TH[:], thr_d[:])
            nc.sync.dma_start(
                MK[:].rearrange("p (q o) -> p q o", q=2),
                mask_d[:].rearrange("q p o -> p q o"),
            )

            # zero whole state tiles once (pads must be 0; data cols are
            # rewritten before every read).  Full-width memsets through an
            # f32-bitcast view — narrow strided bf16 memsets trip a walrus
            # ISA-encoding check (memset_set_value_type).
            for q in range(2):
                for W2 in (Z, WA, WB):
                    nc.vector.memset(W2[q][:, :].bitcast(F32), 0.0)
            nc.vector.memset(R[:, :].bitcast(F32), 0.0)

            def lhsT(kind: int, d: int, q: int):
                i = kind * 2 * ND + d * 2 + q
                return WT[:, i * 128 : (i + 1) * 128]

            def convt_pass(SRC, consume):
                """psum_y[ft] = sum_d Wd^T SRC_blocks; consume(off, size, py)."""
                for off, size in y_tiles:
                    py = py_pool.tile([128, ft], F32, name="pyt", tag="pyt")
                    k = 0
                    for d in range(ND):
                        for q in range(2):
                            lo = BS + off - d
                            nc.tensor.matmul(
                                py[:, :size],
                                lhsT(0, d, q),
                                SRC[q][:, lo : lo + size],
                                start=(k == 0),
                                stop=(k == 2 * ND - 1),
                            )
                            k += 1
                    consume(off, size, py)

            def conv_update(i: int, Wcur, Wold):
                """grad GEMMs + soft-threshold + momentum for iteration i."""
                mom = float(moms[i])
                for ti, (off, size) in enumerate(z_tiles):
                    pgs = []
                    for q in range(2):
                        pg = pg_pool.tile(
                            [128, ft], F32, name=f"pgt{q}", tag="pgt"
                        )
                        for d in range(ND):
                            nc.tensor.matmul(
                                pg[:, :size],
                                lhsT(1, d, q),
                                R[:, off + d : off + d + size],
                                start=(d == 0),
                                stop=(d == ND - 1),
                            )
                        pgs.append(pg)
                    zsl = slice(BS + off, BS + off + size)
                    for q in range(2):
                        if i == 0:
                            # z == 0: w = relu(pg - thr) straight from PSUM
                            nc.scalar.activation(
                                Wcur[q][:, zsl],
                                pgs[q][:, :size],
                                mybir.ActivationFunctionType.Relu,
                                bias=TH[:, 0:1],
                                scale=1.0,
                            )
                        else:
                            t2 = tmps.tile([128, ft], F32, name="t2", tag="t2")
                            nc.vector.tensor_tensor(
                                t2[:, :size],
                                pgs[q][:, :size],
                                Z[q][:, zsl],
                                op=mybir.AluOpType.add,
                            )
                            nc.scalar.activation(
                                Wcur[q][:, zsl],
                                t2[:, :size],
                                mybir.ActivationFunctionType.Relu,
                                bias=TH[:, 0:1],
                                scale=1.0,
                            )
                    if ti == len(z_tiles) - 1:
                        # zero the invalid (s >= Tz%8) lanes of the last block
                        for q in range(2):
                            nc.vector.tensor_scalar(
                                Wcur[q][:, lastz : lastz + 1],
                                Wcur[q][:, lastz : lastz + 1],
                                MK[:, q : q + 1],
                                None,
                                op0=mybir.AluOpType.mult,
                            )
                    if i < n_iter - 1:
                        for q in range(2):
                            if i == 0:
                                nc.vector.tensor_copy(Z[q][:, zsl], Wcur[q][:, zsl])
                            else:
                                dt_ = tmps.tile([128, ft], mmdt, name="dt", tag="dt")
                                nc.vector.tensor_tensor(
                                    dt_[:, :size],
                                    Wcur[q][:, zsl],
                                    Wold[q][:, zsl],
                                    op=mybir.AluOpType.subtract,
                                )
                                nc.vector.scalar_tensor_tensor(
                                    Z[q][:, zsl],
                                    dt_[:, :size],
                                    mom,
                                    Wcur[q][:, zsl],
                                    mybir.AluOpType.mult,
                                    mybir.AluOpType.add,
                                )

            # ---- iteration 0: convt(0) = 0, so r = -x (on ACT) ----
            for off, size in y_tiles:
                nc.scalar.activation(
                    R[:, off : off + size],
                    X[:, off : off + size],
                    mybir.ActivationFunctionType.Copy,
                    bias=0.0,
                    scale=-1.0,
                )
            conv_update(0, WA, WB)
            cur, other = WA, WB

            # ---- iterations 1 .. n_iter-1 ----
            for i in range(1, n_iter):

                def consume_r(off, size, py):
                    nc.vector.tensor_tensor(
                        R[:, off : off + size],
                        py[:, :size],
                        X[:, off : off + size],
                        op=mybir.AluOpType.subtract,
                    )

                convt_pass(Z, consume_r)
                conv_update(i, other, cur)
                cur, other = other, cur

            # ---- output = convt(w_final) ----
            def consume_out(off, size, py):
                nc.scalar.copy(OUT[:, off : off + size], py[:, :size])
                nc.sync.dma_start(out_d[:, off : off + size], OUT[:, off : off + size])

            convt_pass(cur, consume_out)

    nc.compile()
    return nc


_BUILD_CACHE: dict = {}


def _get_nc(T: int, n_iter: int, mm_dtype: str = "bfloat16"):
    key = (T, n_iter, mm_dtype)
    if key not in _BUILD_CACHE:
        _BUILD_CACHE[key] = build_nc(T, n_iter, mm_dtype)
    return _BUILD_CACHE[key]


# test-harness hooks: set TRACE=True before calling kernel() to capture a
# neuron-profile; the measured exec time lands in LAST_RESULT.exec_time_ns.
TRACE = False
LAST_RESULT = None


def kernel(x: np.ndarray, D: np.ndarray) -> np.ndarray:
    from concourse.bass_utils import run_bass_kernel_spmd

    x = np.asarray(x, np.float32)
    D = np.asarray(D, np.float32)
    N, C, T = x.shape
    assert N == 8 and C == CH and D.shape == (KA, CH, KS)

    import ml_dtypes

    L = _lipschitz(D)
    wts = make_weights(D, 1.0 / L).astype(ml_dtypes.bfloat16)
    thrneg = np.full((128, 1), -(LMBD / L), np.float32)

    Tz = T - KS + 1
    NZB = -(-Tz // BS)
    vb = Tz - (NZB - 1) * BS  # valid in-block offsets in the last z block
    masks = np.zeros((2, 128, 1), np.float32)
    for q in range(2):
        for p in range(128):
            masks[q, p, 0] = 1.0 if (4 * q + p // 32) < vb else 0.0

    nc = _get_nc(T, N_ITER)
    in_maps = [
        {"x": block_x(x[n]), "wts": wts, "thrneg": thrneg, "masks": masks}
        for n in range(N)
    ]
    res = run_bass_kernel_spmd(nc, in_maps, list(range(N)), trace=TRACE)
    global LAST_RESULT
    LAST_RESULT = res
    out = np.stack(
        [unblock_out(res.results[i]["out"], T) for i in range(N)], axis=0
    )
    return out.astype(np.float32)


if __name__ == "__main__":
    # smoke: build only
    nc = build_nc(32768, N_ITER)
    print("built OK")


# revision 18
# speedup vs baseline: 1.2517x; 1.2517x over previous
"""Trainium2 Bass kernel for convolutional sparse coding (FISTA, 20 iters).

Problem: x (8, 16, 32768) f32, D (32, 16, 64) f32.
    z = FISTA(x, D)  (20 iterations of grad = conv(convt(z, D) - x, D),
    soft-threshold, Nesterov momentum); output = convt(w_final, D).

Strategy (pure data parallel, one batch element per NeuronCore, D replicated):
  * Block-Toeplitz formulation with time blocks of BS=8: both convolutions
    become 9 GEMM "diagonals" of (128-contraction x 128-out x free) matmuls
    at full 128x128 PE utilization.
  * Layouts: z/w live as 2 SBUF tiles with partition (s%4)*32+k (s = in-block
    time, k = atom); y/x live as 1 tile with partition u*16+c.  The convt
    output layout IS the conv input layout, so no relayout ever happens and
    the whole 20-iteration FISTA state stays resident in SBUF.
  * y-split: grad = conv(convt(z))/L - B with B = conv(x)/L precomputed ONCE
    in bf16 and added into the gradient PSUM via a host-scaled identity
    matmul.  This removes the large exact x term from the per-iteration
    quantized path, so both per-iteration convolutions can run in fp8(e4m3)
    DoubleRow matmuls (2 MACs/cell/cycle, contraction pairs in the AP middle
    dim) with ~3e-3 final relative error.  Iterations >= N_FP8 fall back to
    bf16 matmuls (N_FP8=20 by default: all fp8).
  * Master z/w state is bf16; PSUM accumulates fp32; the soft-threshold
    reads PSUM at fp32.  All data-dependent scales travel as per-partition
    scalar vectors or host-built weight values, so the compiled NEFF depends
    only on shapes.
"""

import os
import sys

import numpy as np

for _p in (
    "/root/.axon_site",
    "/root/.axon_site/_ro/trn_rl_repo",
    "/root/.axon_site/_ro/pypackages",
    "/opt/trn_rl_repo",
):
    if os.path.isdir(_p) and _p not in sys.path:
        sys.path.append(_p)

import concourse.bass as bass  # noqa: E402
import concourse.tile as tile  # noqa: E402
from concourse import bacc, mybir  # noqa: E402

BS = 8  # time-block size
KS = 64  # conv kernel length
ND = KS // BS + 1  # 9 gemm diagonals
NPR = (ND + 1) // 2  # 5 fp8 DoubleRow diagonal-pairs
KA = 32  # dictionary atoms
CH = 16  # channels
LMBD = 0.1
N_ITER = 20
N_FP8 = 20  # iterations 0..N_FP8-1 use fp8 DoubleRow; rest bf16
FT = 512  # free-dim (block-index) tile size == one PSUM bank of fp32

SY = 32.0  # y is stored scaled by SY in fp8 mode
SW = 64.0  # fp8 convt weight pre-scale (|D| <= 1 so SW*|D| <= 64 << 448)
CPOW = float(2 ** 21)  # gradient-PSUM scale; exact in bf16 (identity value)

F32 = mybir.dt.float32
BF16 = mybir.dt.bfloat16
FP8 = mybir.dt.float8e4

# bf16 weight-set indices: [kind, d, q] with kinds
#   0: convt W (D), 1: conv V (-D/(SY*L)), 2: B-precompute VB (+D/L)
NW_BF = 3 * ND * 2 + 2  # + identity tiles: [54] = diag(SW*SY*L), [55] = diag(1)
IDX_IC = 3 * ND * 2
IDX_I1 = 3 * ND * 2 + 1


def _lipschitz(D: np.ndarray) -> float:
    F = np.fft.fft(D.astype(np.float64), axis=2)
    L = float(np.sum(np.max((F * np.conj(F)).real, axis=2)))
    return 1.0 if L == 0.0 else L


def _momentum_schedule(n_iter: int) -> list[float]:
    betas = [1.0]
    for _ in range(n_iter):
        b = betas[-1]
        betas.append((1.0 + (1.0 + 4.0 * b * b) ** 0.5) / 2.0)
    return [(betas[i] - 1.0) / betas[i + 1] for i in range(n_iter)]


def _toeplitz_entry(D, d, u, s):
    j = BS * d + u - s
    if 0 <= j < KS:
        return D[:, :, j]  # [k, c]
    return None


def make_weights(D: np.ndarray, L: float) -> np.ndarray:
    """bf16 weights, shape (NW_BF, 128, 128), fp32 (cast later)."""
    D = np.asarray(D, np.float32)
    wts = np.zeros((NW_BF, 128, 128), np.float32)
    for kind, cs in ((0, 1.0), (1, -1.0 / (SY * L)), (2, 1.0 / L)):
        for d in range(ND):
            for q in range(2):
                M = np.zeros((4, KA, BS, CH), np.float32)  # (sl, k, u, c)
                V = np.zeros((BS, CH, 4, KA), np.float32)  # (u, c, sl, k)
                for sl in range(4):
                    s = 4 * q + sl
                    for u in range(BS):
                        e = _toeplitz_entry(D, d, u, s)
                        if e is not None:
                            M[sl, :, u, :] = e
                            V[u, :, sl, :] = cs * e.T
                wts[kind * 2 * ND + d * 2 + q] = (
                    M.reshape(128, 128) if kind == 0 else V.reshape(128, 128)
                )
    wts[IDX_IC] = np.eye(128, dtype=np.float32) * CPOW
    wts[IDX_I1] = np.eye(128, dtype=np.float32)
    return wts


def make_weights8(D: np.ndarray, L: float) -> np.ndarray:
    """fp8 DoubleRow weights, shape (ND + 2*NPR, 128, 2, 128), float8e4.

    convt tiles carry SW*D; conv tiles carry -(CPOW/(SY*L))*D so that the
    gradient PSUM is exactly CPOW*(z + B - conv(y)/L) with the z/B identity
    matmuls using the bf16-exact value CPOW."""
    import ml_dtypes

    SWC = CPOW / (SY * L)
    D = np.asarray(D, np.float32)
    w8 = np.zeros((ND + 2 * NPR, 128, 2, 128), np.float32)
    for d in range(ND):
        for j in range(2):
            M = np.zeros((4, KA, BS, CH), np.float32)
            for sl in range(4):
                for u in range(BS):
                    e = _toeplitz_entry(D, d, u, 4 * j + sl)
                    if e is not None:
                        M[sl, :, u, :] = SW * e
            w8[d, :, j, :] = M.reshape(128, 128)
    for pr in range(NPR):
        for q in range(2):
            for jj in range(2):
                d = 2 * pr + jj
                if d >= ND:
                    continue
                V = np.zeros((BS, CH, 4, KA), np.float32)
                for sl in range(4):
                    for u in range(BS):
                        e = _toeplitz_entry(D, d, u, 4 * q + sl)
                        if e is not None:
                            V[u, :, sl, :] = -SWC * e.T
                w8[ND + pr * 2 + q, :, jj, :] = V.reshape(128, 128)
    return w8.astype(ml_dtypes.float8_e4m3)


def block_x(xn: np.ndarray) -> np.ndarray:
    """(C, T) -> (128, T//BS) with partition u*16+c, col b = x[c, 8b+u]."""
    C, T = xn.shape
    return (
        np.ascontiguousarray(
            xn.reshape(C, T // BS, BS).transpose(2, 0, 1)
        ).reshape(128, T // BS)
    )


def unblock_out(ob: np.ndarray, T: int) -> np.ndarray:
    """(128, T//BS) -> (C, T)."""
    return np.ascontiguousarray(
        ob.reshape(BS, CH, T // BS).transpose(1, 2, 0)
    ).reshape(CH, T)


def build_nc(T: int, n_iter: int, n_fp8: int = N_FP8, ft: int = FT):
    """Build + compile the single-core FISTA graph (SPMD across 8 cores).

    DRAM params (host-prepped):
      x      (128, NB) bf16  pre-blocked x (B-precompute GEMM input)
      wts    (NW_BF, 128, 128) bf16
      wts8   (ND+2*NPR, 128, 2, 128) fp8e4
      scal   (128, 4) f32  per-partition [-thr, ay, b8, Sz] columns
      masks  (2, 128, 1) f32
      out    (128, NB) f32
    with Sz = L, ay = SY/(SW*Sz), b8 = 1/(SW*SY*L).
    """
    assert T % BS == 0
    NB = T // BS
    Tz = T - KS + 1
    NZB = -(-Tz // BS)
    ZC = NB + BS  # 8 left zero-pad + NZB data + right zero-pad
    YC = NB + 2  # one zero halo block (+1 spare: keep bytes %4)
    YC8 = -(-YC // 16) * 16  # fp8 plane tile: middle-dim step %16 bytes
    y_tiles = [(i * ft, min(ft, NB - i * ft)) for i in range(-(-NB // ft))]
    z_tiles = [(i * ft, min(ft, NZB - i * ft)) for i in range(-(-NZB // ft))]
    moms = _momentum_schedule(n_iter)
    lastz = BS + NZB - 1  # column of the (partially valid) last z block
    DR = mybir.MatmulPerfMode.DoubleRow
    AOP = mybir.AluOpType

    nc = bacc.Bacc("TRN2", target_bir_lowering=False, debug=False)
    x_d = nc.declare_dram_parameter("x", [128, NB], BF16, isOutput=False)
    wts_d = nc.declare_dram_parameter("wts", [NW_BF, 128, 128], BF16, isOutput=False)
    wts8_d = nc.declare_dram_parameter(
        "wts8", [ND + 2 * NPR, 128, 2, 128], FP8, isOutput=False
    )
    scal_d = nc.declare_dram_parameter("scal", [128, 4], F32, isOutput=False)
    mask_d = nc.declare_dram_parameter("masks", [2, 128, 1], F32, isOutput=False)
    out_d = nc.declare_dram_parameter("out", [128, NB], F32, isOutput=True)

    with tile.TileContext(nc) as tc:
        with (
            tc.tile_pool(name="state", bufs=1) as state,
            tc.tile_pool(name="py", bufs=2, space="PSUM") as py_pool,
            tc.tile_pool(name="pg", bufs=4, space="PSUM") as pg_pool,
            tc.tile_pool(name="tmps", bufs=4) as tmps,
        ):
            XB = state.tile([128, YC], BF16, name="XB")
            YF = state.tile([128, YC], BF16, name="YF")
            Y2 = state.tile([128, 2, YC8], FP8, name="Y2")
            Z = [state.tile([128, ZC], BF16, name=f"Zq{q}") for q in range(2)]
            Z8 = state.tile([128, 2, -(-ZC // 16) * 16], FP8, name="Z8")
            B = [state.tile([128, ZC], BF16, name=f"Bq{q}") for q in range(2)]
            WA = [state.tile([128, ZC], BF16, name=f"WAq{q}") for q in range(2)]
            WB = [state.tile([128, ZC], BF16, name=f"WBq{q}") for q in range(2)]
            WT = state.tile([128, NW_BF * 128], BF16, name="WT")
            WT8 = state.tile([128, ND + 2 * NPR, 2, 128], FP8, name="WT8")
            SC = state.tile([128, 4], F32, name="SC")
            MK = state.tile([128, 2], F32, name="MK")
            OUT = state.tile([128, NB], F32, name="OUTS")

            nc.sync.dma_start(XB[:, :NB], x_d[:])
            nc.sync.dma_start(
                WT[:].rearrange("p (i j) -> p i j", i=NW_BF),
                wts_d[:].rearrange("i p j -> p i j"),
            )
            nc.sync.dma_start(WT8[:], wts8_d[:].rearrange("i p j c -> p i j c"))
            nc.sync.dma_start(SC[:], scal_d[:])
            nc.sync.dma_start(
                MK[:].rearrange("p (q o) -> p q o", q=2),
                mask_d[:].rearrange("q p o -> p q o"),
            )
            TH = SC[:, 0:1]  # -thr
            AY = SC[:, 1:2]  # SY/(SW*Sz)
            B8 = SC[:, 2:3]  # 1/(SW*SY*L)
            SZ = SC[:, 3:4]  # Sz

            # zero pads / halos once via f32-bitcast full-tile memsets
            nc.vector.memset(XB[:, NB:YC].bitcast(F32), 0.0)
            for q in range(2):
                for W2 in (Z, WA, WB):
                    nc.vector.memset(W2[q][:, :].bitcast(F32), 0.0)
            nc.vector.memset(YF[:, :].bitcast(F32), 0.0)
            nc.vector.memset(Y2[:, :, :].bitcast(F32), 0.0)
            nc.vector.memset(Z8[:, :, :].bitcast(F32), 0.0)

            def lhsT(kind: int, d: int, q: int):
                i = kind * 2 * ND + d * 2 + q
                return WT[:, i * 128 : (i + 1) * 128]

            I_C = WT[:, IDX_IC * 128 : (IDX_IC + 1) * 128]
            I_1 = WT[:, IDX_I1 * 128 : (IDX_I1 + 1) * 128]

            # ---- B = conv(x)/L in z-layout (one-time, bf16) ----
            for off, size in z_tiles:
                for q in range(2):
                    pb = pg_pool.tile([128, ft], F32, name="pbt", tag="pgt")
                    for d in range(ND):
                        nc.tensor.matmul(
                            pb[:, :size],
                            lhsT(2, d, q),
                            XB[:, off + d : off + d + size],
                            start=(d == 0),
                            stop=(d == ND - 1),
                        )
                    nc.scalar.copy(B[q][:, BS + off : BS + off + size], pb[:, :size])

            def convt_pass(SRC, consume):
                """bf16: psum_y[ft] = sum_d Wd^T SRC_blocks (= y)."""
                for off, size in y_tiles:
                    py = py_pool.tile([128, ft], F32, name="pyt", tag="pyt")
                    k = 0
                    for d in range(ND):
                        for q in range(2):
                            lo = BS + off - d
                            nc.tensor.matmul(
                                py[:, :size],
                                lhsT(0, d, q),
                                SRC[q][:, lo : lo + size],
                                start=(k == 0),
                                stop=(k == 2 * ND - 1),
                            )
                            k += 1
                    consume(off, size, py)

            def convt_pass8(consume):
                """fp8 DR: psum_y[ft] = SW*Sz*y from Z8."""
                for off, size in y_tiles:
                    py = py_pool.tile([128, ft], F32, name="pyt", tag="pyt")
                    for d in range(ND):
                        lo = BS + off - d
                        nc.tensor.matmul(
                            py[:, :size],
                            WT8[:, d, :, :],
                            Z8[:, :, lo : lo + size],
                            start=(d == 0),
                            stop=(d == ND - 1),
                            perf_mode=DR,
                        )
                    consume(off, size, py)

            def write_y(off, size, py, fp8: bool):
                """Y2 planes (= SY*y fp8) or YF (= SY*y bf16) from psum."""
                if fp8:
                    # psum = SW*Sz*y -> y8 = ay*psum  (on ACT)
                    nc.scalar.activation(
                        Y2[:, 0, off : off + size],
                        py[:, :size],
                        mybir.ActivationFunctionType.Copy,
                        bias=0.0,
                        scale=AY,
                    )
                    # plane1 = plane0 shifted left by one block; plane1[NB-1]
                    # stays 0 from the initial memset (= zero halo)
                    dlo = max(off - 1, 0)
                    nc.vector.tensor_copy(
                        Y2[:, 1, dlo : off + size - 1],
                        Y2[:, 0, dlo + 1 : off + size],
                    )
                else:
                    # psum = y -> yf = SY*psum
                    nc.vector.tensor_scalar(
                        YF[:, off : off + size],
                        py[:, :size],
                        float(SY),
                        None,
                        op0=AOP.mult,
                    )

            def conv_update(i: int, Wcur, Wold):
                """gradient GEMMs + B + soft-threshold + momentum, iter i."""
                fp8 = i < n_fp8
                next_fp8 = (i + 1) < n_fp8
                mom = float(moms[i])
                for ti, (off, size) in enumerate(z_tiles):
                    pgs = []
                    for q in range(2):
                        if i == 0:
                            pgs.append(None)  # t2 comes straight from B
                            continue
                        pg = pg_pool.tile([128, ft], F32, name=f"pgt{q}", tag="pgt")
                        bsl = B[q][:, BS + off : BS + off + size]
                        zslq = Z[q][:, BS + off : BS + off + size]
                        if fp8:
                            for pr in range(NPR):
                                nc.tensor.matmul(
                                    pg[:, :size],
                                    WT8[:, ND + pr * 2 + q, :, :],
                                    Y2[:, :, off + 2 * pr : off + 2 * pr + size],
                                    start=(pr == 0),
                                    stop=False,
                                    perf_mode=DR,
                                )
                            # += (SW*SY*L)*(B + z): b8*psum = z - conv(y)/L + B
                            nc.tensor.matmul(
                                pg[:, :size], I_C, bsl, start=False, stop=False
                            )
                            nc.tensor.matmul(
                                pg[:, :size], I_C, zslq, start=False, stop=True
                            )
                        else:
                            for d in range(ND):
                                nc.tensor.matmul(
                                    pg[:, :size],
                                    lhsT(1, d, q),
                                    YF[:, off + d : off + d + size],
                                    start=(d == 0),
                                    stop=False,
                                )
                            nc.tensor.matmul(
                                pg[:, :size], I_1, bsl, start=False, stop=False
                            )
                            nc.tensor.matmul(
                                pg[:, :size], I_1, zslq, start=False, stop=True
                            )
                        pgs.append(pg)
                    zsl = slice(BS + off, BS + off + size)
                    for q in range(2):
                        if i == 0:
                            # z == 0, y == 0: w = relu(B - thr)
                            nc.scalar.activation(
                                Wcur[q][:, zsl],
                                B[q][:, zsl],
                                mybir.ActivationFunctionType.Relu,
                                bias=TH,
                                scale=1.0,
                            )
                        else:
                            nc.scalar.activation(
                                Wcur[q][:, zsl],
                                pgs[q][:, :size],
                                mybir.ActivationFunctionType.Relu,
                                bias=TH,
                                scale=B8 if fp8 else 1.0,
                            )
                    if ti == len(z_tiles) - 1:
                        # zero the invalid (s >= Tz%8) lanes of the last block
                        for q in range(2):
                            nc.vector.tensor_scalar(
                                Wcur[q][:, lastz : lastz + 1],
                                Wcur[q][:, lastz : lastz + 1],
                                MK[:, q : q + 1],
                                None,
                                op0=AOP.mult,
                            )
                    if i < n_iter - 1:
                        # momentum + fp8-z in coarse chunks, emitted as soon
                        # as the covering tiles' w is final (keeps the next
                        # iteration's convt dependencies early in each
                        # engine's FIFO)
                        CH_COLS = 2 * ft
                        c0 = ti // 2 * CH_COLS if ti % 2 == 1 else None
                        if ti == len(z_tiles) - 1 and len(z_tiles) % 2 == 1:
                            c0 = (len(z_tiles) - 1) // 2 * CH_COLS
                        if c0 is None or c0 >= NZB:
                            continue
                        cs = min(CH_COLS, NZB - c0)
                        csl = slice(BS + c0, BS + c0 + cs)
                        for q in range(2):
                            if i == 0:
                                nc.vector.tensor_copy(Z[q][:, csl], Wcur[q][:, csl])
                            else:
                                dt_ = tmps.tile(
                                    [128, CH_COLS], BF16, name="dt", tag="dt"
                                )
                                nc.vector.tensor_tensor(
                                    dt_[:, :cs],
                                    Wcur[q][:, csl],
                                    Wold[q][:, csl],
                                    op=AOP.subtract,
                                )
                                nc.vector.scalar_tensor_tensor(
                                    Z[q][:, csl],
                                    dt_[:, :cs],
                                    mom,
                                    Wcur[q][:, csl],
                                    AOP.mult,
                                    AOP.add,
                                )
                            if next_fp8:
                                # fp8 copy of z for next iter (ACT): Sz * Z
                                nc.scalar.activation(
                                    Z8[:, q, csl],
                                    Z[q][:, csl],
                                    mybir.ActivationFunctionType.Copy,
                                    bias=0.0,
                                    scale=SZ,
                                )

            # ---- iteration 0 (y = 0: w1 = relu(B - thr), no GEMMs) ----
            conv_update(0, WA, WB)
            cur, other = WA, WB

            # ---- iterations 1 .. n_iter-1 ----
            for i in range(1, n_iter):
                fp8 = i < n_fp8

                def consume_y(off, size, py, _fp8=fp8):
                    write_y(off, size, py, _fp8)

                if fp8:
                    convt_pass8(consume_y)
                else:
                    convt_pass(Z, consume_y)
                conv_update(i, other, cur)
                cur, other = other, cur

            # ---- output = convt(w_final), always bf16 ----
            def consume_out(off, size, py):
                nc.scalar.copy(OUT[:, off : off + size], py[:, :size])
                nc.sync.dma_start(out_d[:, off : off + size], OUT[:, off : off + size])

            convt_pass(cur, consume_out)

    nc.compile()
    return nc


_BUILD_CACHE: dict = {}


def _get_nc(T: int, n_iter: int, n_fp8: int = N_FP8):
    key = (T, n_iter, n_fp8)
    if key not in _BUILD_CACHE:
        _BUILD_CACHE[key] = build_nc(T, n_iter, n_fp8)
    return _BUILD_CACHE[key]


def host_inputs(D: np.ndarray, T: int):
    """All data-dependent device inputs except x (shared across cores)."""
    import ml_dtypes

    L = _lipschitz(D)
    Sz = L
    wts = make_weights(D, L).astype(ml_dtypes.bfloat16)
    wts8 = make_weights8(D, L)
    scal = np.zeros((128, 4), np.float32)
    scal[:, 0] = -(LMBD / L)
    scal[:, 1] = SY / (SW * Sz)
    scal[:, 2] = 1.0 / CPOW
    scal[:, 3] = Sz
    Tz = T - KS + 1
    NZB = -(-Tz // BS)
    vb = Tz - (NZB - 1) * BS
    masks = np.zeros((2, 128, 1), np.float32)
    for q in range(2):
        for p in range(128):
            masks[q, p, 0] = 1.0 if (4 * q + p // 32) < vb else 0.0
    return {"wts": wts, "wts8": wts8, "scal": scal, "masks": masks}


# test-harness hooks: set TRACE=True before calling kernel() to capture a
# neuron-profile; the measured exec time lands in LAST_RESULT.exec_time_ns.
TRACE = False
LAST_RESULT = None


def kernel(x: np.ndarray, D: np.ndarray) -> np.ndarray:
    import ml_dtypes

    from concourse.bass_utils import run_bass_kernel_spmd

    x = np.asarray(x, np.float32)
    D = np.asarray(D, np.float32)
    N, C, T = x.shape
    assert N == 8 and C == CH and D.shape == (KA, CH, KS)

    shared = host_inputs(D, T)
    nc = _get_nc(T, N_ITER, N_FP8)
    in_maps = [
        {"x": block_x(x[n]).astype(ml_dtypes.bfloat16), **shared} for n in range(N)
    ]
    res = run_bass_kernel_spmd(nc, in_maps, list(range(N)), trace=TRACE)
    global LAST_RESULT
    LAST_RESULT = res
    out = np.stack(
        [
            unblock_out(np.asarray(res.results[i]["out"], np.float32), T)
            for i in range(N)
        ],
        axis=0,
    )
    return out.astype(np.float32)


if __name__ == "__main__":
    nc = build_nc(32768, N_ITER)
    print("built OK")


# revision 24
# speedup vs baseline: 1.3787x; 1.1014x over previous
"""Trainium2 Bass kernel for convolutional sparse coding (FISTA, 20 iters).

Problem: x (8, 16, 32768) f32, D (32, 16, 64) f32.
    z = FISTA(x, D)  (20 iterations of grad = conv(convt(z, D) - x, D),
    soft-threshold, Nesterov momentum); output = convt(w_final, D).

Strategy (pure data parallel, one batch element per NeuronCore, D replicated):
  * Block-Toeplitz formulation with time blocks of BS=8: both convolutions
    become 9 GEMM "diagonals" of (128-contraction x 128-out x free) matmuls
    at full 128x128 PE utilization.
  * Layouts: z/w live as 2 SBUF tiles with partition (s%4)*32+k (s = in-block
    time, k = atom); y/x live as 1 tile with partition u*16+c.  The convt
    output layout IS the conv input layout, so no relayout ever happens and
    the whole 20-iteration FISTA state stays resident in SBUF.
  * y-split: grad = conv(convt(z))/L - B with B = conv(x)/L precomputed ONCE
    in bf16 and added into the gradient PSUM via a host-scaled identity
    matmul.  This removes the large exact x term from the per-iteration
    quantized path, so both per-iteration convolutions can run in fp8(e4m3)
    DoubleRow matmuls (2 MACs/cell/cycle, contraction pairs in the AP middle
    dim) with ~3e-3 final relative error.  Iterations >= N_FP8 fall back to
    bf16 matmuls (N_FP8=20 by default: all fp8).
  * Master z/w state is bf16; PSUM accumulates fp32; the soft-threshold
    reads PSUM at fp32.  All data-dependent scales travel as per-partition
    scalar vectors or host-built weight values, so the compiled NEFF depends
    only on shapes.
"""

import os
import sys

import numpy as np

for _p in (
    "/root/.axon_site",
    "/root/.axon_site/_ro/trn_rl_repo",
    "/root/.axon_site/_ro/pypackages",
    "/opt/trn_rl_repo",
):
    if os.path.isdir(_p) and _p not in sys.path:
        sys.path.append(_p)

import concourse.bass as bass  # noqa: E402
import concourse.tile as tile  # noqa: E402
from concourse import bacc, mybir  # noqa: E402

BS = 8  # time-block size
KS = 64  # conv kernel length
ND = KS // BS + 1  # 9 gemm diagonals
NPR = (ND + 1) // 2  # 5 fp8 DoubleRow diagonal-pairs
KA = 32  # dictionary atoms
CH = 16  # channels
LMBD = 0.1
N_ITER = 20
N_FP8 = 20  # iterations 0..N_FP8-1 use fp8 DoubleRow; rest bf16
FT = 512  # free-dim (block-index) tile size == one PSUM bank of fp32

SY = 32.0  # y is stored scaled by SY in fp8 mode
SW = 64.0  # fp8 convt weight pre-scale (|D| <= 1 so SW*|D| <= 64 << 448)
CPOW = float(2 ** 21)  # gradient-PSUM scale; exact in bf16 (identity value)

F32 = mybir.dt.float32
BF16 = mybir.dt.bfloat16
FP8 = mybir.dt.float8e4

# bf16 weight-set indices: [kind, d, q] with kinds
#   0: convt W (D), 1: conv V (-D/(SY*L)), 2: B-precompute VB (+D/L)
NW_BF = 3 * ND * 2 + 2  # + identity tiles: [54] = diag(SW*SY*L), [55] = diag(1)
IDX_IC = 3 * ND * 2
IDX_I1 = 3 * ND * 2 + 1


def _lipschitz(D: np.ndarray) -> float:
    F = np.fft.fft(D.astype(np.float64), axis=2)
    L = float(np.sum(np.max((F * np.conj(F)).real, axis=2)))
    return 1.0 if L == 0.0 else L


def _momentum_schedule(n_iter: int) -> list[float]:
    betas = [1.0]
    for _ in range(n_iter):
        b = betas[-1]
        betas.append((1.0 + (1.0 + 4.0 * b * b) ** 0.5) / 2.0)
    return [(betas[i] - 1.0) / betas[i + 1] for i in range(n_iter)]


def _toeplitz_entry(D, d, u, s):
    j = BS * d + u - s
    if 0 <= j < KS:
        return D[:, :, j]  # [k, c]
    return None


def make_weights(D: np.ndarray, L: float) -> np.ndarray:
    """bf16 weights, shape (NW_BF, 128, 128), fp32 (cast later)."""
    D = np.asarray(D, np.float32)
    wts = np.zeros((NW_BF, 128, 128), np.float32)
    for kind, cs in ((0, 1.0), (1, -1.0 / (SY * L)), (2, 1.0 / L)):
        for d in range(ND):
            for q in range(2):
                M = np.zeros((4, KA, BS, CH), np.float32)  # (sl, k, u, c)
                V = np.zeros((BS, CH, 4, KA), np.float32)  # (u, c, sl, k)
                for sl in range(4):
                    s = 4 * q + sl
                    for u in range(BS):
                        e = _toeplitz_entry(D, d, u, s)
                        if e is not None:
                            M[sl, :, u, :] = e
                            V[u, :, sl, :] = cs * e.T
                wts[kind * 2 * ND + d * 2 + q] = (
                    M.reshape(128, 128) if kind == 0 else V.reshape(128, 128)
                )
    wts[IDX_IC] = np.eye(128, dtype=np.float32) * CPOW
    wts[IDX_I1] = np.eye(128, dtype=np.float32)
    return wts


def make_weights8(D: np.ndarray, L: float) -> np.ndarray:
    """fp8 DoubleRow weights, shape (ND + 2*NPR, 128, 2, 128), float8e4.

    convt tiles carry SW*D; conv tiles carry -(CPOW/(SY*L))*D so that the
    gradient PSUM is exactly CPOW*(z + B - conv(y)/L) with the z/B identity
    matmuls using the bf16-exact value CPOW."""
    import ml_dtypes

    SWC = CPOW / (SY * L)
    D = np.asarray(D, np.float32)
    w8 = np.zeros((ND + 2 * NPR, 128, 2, 128), np.float32)
    for d in range(ND):
        for j in range(2):
            M = np.zeros((4, KA, BS, CH), np.float32)
            for sl in range(4):
                for u in range(BS):
                    e = _toeplitz_entry(D, d, u, 4 * j + sl)
                    if e is not None:
                        M[sl, :, u, :] = SW * e
            w8[d, :, j, :] = M.reshape(128, 128)
    for pr in range(NPR):
        for q in range(2):
            for jj in range(2):
                d = 2 * pr + jj
                if d >= ND:
                    continue
                V = np.zeros((BS, CH, 4, KA), np.float32)
                for sl in range(4):
                    for u in range(BS):
                        e = _toeplitz_entry(D, d, u, 4 * q + sl)
                        if e is not None:
                            V[u, :, sl, :] = -SWC * e.T
                w8[ND + pr * 2 + q, :, jj, :] = V.reshape(128, 128)
    return w8.astype(ml_dtypes.float8_e4m3)


def block_x(xn: np.ndarray) -> np.ndarray:
    """(C, T) -> (128, T//BS) with partition u*16+c, col b = x[c, 8b+u]."""
    C, T = xn.shape
    return (
        np.ascontiguousarray(
            xn.reshape(C, T // BS, BS).transpose(2, 0, 1)
        ).reshape(128, T // BS)
    )


def unblock_out(ob: np.ndarray, T: int) -> np.ndarray:
    """(128, T//BS) -> (C, T)."""
    return np.ascontiguousarray(
        ob.reshape(BS, CH, T // BS).transpose(1, 2, 0)
    ).reshape(CH, T)


def build_nc(T: int, n_iter: int, n_fp8: int = N_FP8, ft: int = FT):
    """Build + compile the single-core FISTA graph (SPMD across 8 cores).

    DRAM params (host-prepped):
      x      (128, NB) bf16  pre-blocked x (B-precompute GEMM input)
      wts    (NW_BF, 128, 128) bf16
      wts8   (ND+2*NPR, 128, 2, 128) fp8e4
      scal   (128, 4) f32  per-partition [-thr, ay, b8, Sz] columns
      masks  (2, 128, 1) f32
      out    (128, NB) f32
    with Sz = L, ay = SY/(SW*Sz), b8 = 1/(SW*SY*L).
    """
    assert T % BS == 0
    NB = T // BS
    Tz = T - KS + 1
    NZB = -(-Tz // BS)
    ZC = NB + BS  # 8 left zero-pad + NZB data + right zero-pad
    YC = NB + 2  # one zero halo block (+1 spare: keep bytes %4)
    YC8 = -(-YC // 16) * 16  # fp8 plane tile: middle-dim step %16 bytes
    y_tiles = [(i * ft, min(ft, NB - i * ft)) for i in range(-(-NB // ft))]
    z_tiles = [(i * ft, min(ft, NZB - i * ft)) for i in range(-(-NZB // ft))]
    moms = _momentum_schedule(n_iter)
    lastz = BS + NZB - 1  # column of the (partially valid) last z block
    DR = mybir.MatmulPerfMode.DoubleRow
    AOP = mybir.AluOpType

    nc = bacc.Bacc("TRN2", target_bir_lowering=False, debug=False)
    x_d = nc.declare_dram_parameter("x", [128, NB], BF16, isOutput=False)
    wts_d = nc.declare_dram_parameter("wts", [NW_BF, 128, 128], BF16, isOutput=False)
    wts8_d = nc.declare_dram_parameter(
        "wts8", [ND + 2 * NPR, 128, 2, 128], FP8, isOutput=False
    )
    scal_d = nc.declare_dram_parameter(
        "scal", [128, 7 + n_iter], F32, isOutput=False
    )
    mask_d = nc.declare_dram_parameter("masks", [2, 128, 1], F32, isOutput=False)
    out_d = nc.declare_dram_parameter("out", [128, NB], F32, isOutput=True)

    with tile.TileContext(nc) as tc:
        with (
            tc.tile_pool(name="state", bufs=1) as state,
            tc.tile_pool(name="py", bufs=2, space="PSUM") as py_pool,
            tc.tile_pool(name="pg", bufs=4, space="PSUM") as pg_pool,
            tc.tile_pool(name="tmps", bufs=4) as tmps,
        ):
            XB = state.tile([128, YC], BF16, name="XB")
            YF = state.tile([128, YC], BF16, name="YF")
            Y2 = state.tile([128, 2, YC8], FP8, name="Y2")
            Z = [state.tile([128, ZC], BF16, name=f"Zq{q}") for q in range(2)]
            ZB = [state.tile([128, ZC], BF16, name=f"ZBq{q}") for q in range(2)]
            Z8 = state.tile([128, 2, -(-ZC // 16) * 16], FP8, name="Z8")
            B = [state.tile([128, ZC], BF16, name=f"Bq{q}") for q in range(2)]
            WA = [state.tile([128, ZC], BF16, name=f"WAq{q}") for q in range(2)]
            WB = [state.tile([128, ZC], BF16, name=f"WBq{q}") for q in range(2)]
            WT = state.tile([128, NW_BF * 128], BF16, name="WT")
            WT8 = state.tile([128, ND + 2 * NPR, 2, 128], FP8, name="WT8")
            SC = state.tile([128, 7 + n_iter], F32, name="SC")
            MK = state.tile([128, 2], F32, name="MK")
            OUT = state.tile([128, NB], F32, name="OUTS")

            nc.sync.dma_start(XB[:, :NB], x_d[:])
            nc.sync.dma_start(
                WT[:].rearrange("p (i j) -> p i j", i=NW_BF),
                wts_d[:].rearrange("i p j -> p i j"),
            )
            nc.sync.dma_start(WT8[:], wts8_d[:].rearrange("i p j c -> p i j c"))
            nc.sync.dma_start(SC[:], scal_d[:])
            nc.sync.dma_start(
                MK[:].rearrange("p (q o) -> p q o", q=2),
                mask_d[:].rearrange("q p o -> p q o"),
            )
            TH = SC[:, 0:1]  # -thr
            AY = SC[:, 1:2]  # SY/(SW*Sz)
            B8 = SC[:, 2:3]  # 1/CPOW
            SZ = SC[:, 3:4]  # Sz
            BS8 = SC[:, 4:5]  # Sz/CPOW (relu scale for scaled-w state)
            ISZ = SC[:, 5:6]  # 1/Sz
            NLM = SC[:, 6:7]  # -LMBD (= -Sz*thr, bias in scaled-w space)

            # zero pads / halos once via f32-bitcast full-tile memsets
            nc.vector.memset(XB[:, NB:YC].bitcast(F32), 0.0)
            for q in range(2):
                for W2 in (Z, ZB, WA, WB):
                    nc.vector.memset(W2[q][:, :].bitcast(F32), 0.0)
            nc.vector.memset(YF[:, :].bitcast(F32), 0.0)
            nc.vector.memset(Y2[:, :, :].bitcast(F32), 0.0)
            nc.vector.memset(Z8[:, :, :].bitcast(F32), 0.0)

            def lhsT(kind: int, d: int, q: int):
                i = kind * 2 * ND + d * 2 + q
                return WT[:, i * 128 : (i + 1) * 128]

            I_C = WT[:, IDX_IC * 128 : (IDX_IC + 1) * 128]
            I_1 = WT[:, IDX_I1 * 128 : (IDX_I1 + 1) * 128]

            # ---- B = conv(x)/L in z-layout (one-time, bf16) ----
            for off, size in z_tiles:
                for q in range(2):
                    pb = pg_pool.tile([128, ft], F32, name="pbt", tag="pgt")
                    for d in range(ND):
                        nc.tensor.matmul(
                            pb[:, :size],
                            lhsT(2, d, q),
                            XB[:, off + d : off + d + size],
                            start=(d == 0),
                            stop=(d == ND - 1),
                        )
                    nc.scalar.copy(B[q][:, BS + off : BS + off + size], pb[:, :size])

            def convt_pass(SRC, consume):
                """bf16: psum_y[ft] = sum_d Wd^T SRC_blocks (= y)."""
                for off, size in y_tiles:
                    py = py_pool.tile([128, ft], F32, name="pyt", tag="pyt")
                    k = 0
                    for d in range(ND):
                        for q in range(2):
                            lo = BS + off - d
                            nc.tensor.matmul(
                                py[:, :size],
                                lhsT(0, d, q),
                                SRC[q][:, lo : lo + size],
                                start=(k == 0),
                                stop=(k == 2 * ND - 1),
                            )
                            k += 1
                    consume(off, size, py)

            def convt_pass8(consume):
                """fp8 DR: psum_y[ft] = SW*Sz*y from Z8."""
                for off, size in y_tiles:
                    py = py_pool.tile([128, ft], F32, name="pyt", tag="pyt")
                    for d in range(ND):
                        lo = BS + off - d
                        nc.tensor.matmul(
                            py[:, :size],
                            WT8[:, d, :, :],
                            Z8[:, :, lo : lo + size],
                            start=(d == 0),
                            stop=(d == ND - 1),
                            perf_mode=DR,
                        )
                    consume(off, size, py)

            def write_y(off, size, py, fp8: bool):
                """Y2 planes (= SY*y fp8) or YF (= SY*y bf16) from psum."""
                if fp8:
                    # psum = SW*Sz*y -> y8 = ay*psum  (on ACT)
                    nc.scalar.activation(
                        Y2[:, 0, off : off + size],
                        py[:, :size],
                        mybir.ActivationFunctionType.Copy,
                        bias=0.0,
                        scale=AY,
                    )
                    # plane1 = plane0 shifted left by one block; plane1[NB-1]
                    # stays 0 from the initial memset (= zero halo)
                    dlo = max(off - 1, 0)
                    nc.vector.tensor_copy(
                        Y2[:, 1, dlo : off + size - 1],
                        Y2[:, 0, dlo + 1 : off + size],
                    )
                else:
                    # psum = y -> yf = SY*psum
                    nc.vector.tensor_scalar(
                        YF[:, off : off + size],
                        py[:, :size],
                        float(SY),
                        None,
                        op0=AOP.mult,
                    )

            def conv_update(i: int, Wcur, Wold):
                """gradient GEMMs + B + soft-threshold + momentum, iter i."""
                fp8 = i < n_fp8
                next_fp8 = (i + 1) < n_fp8
                mom = float(moms[i])
                for ti, (off, size) in enumerate(z_tiles):
                    pgs = []
                    for q in range(2):
                        if i == 0:
                            pgs.append(None)  # t2 comes straight from B
                            continue
                        pg = pg_pool.tile([128, ft], F32, name=f"pgt{q}", tag="pgt")
                        bsl = B[q][:, BS + off : BS + off + size]
                        zbsl = ZB[q][:, BS + off : BS + off + size]
                        if fp8:
                            for pr in range(NPR):
                                nc.tensor.matmul(
                                    pg[:, :size],
                                    WT8[:, ND + pr * 2 + q, :, :],
                                    Y2[:, :, off + 2 * pr : off + 2 * pr + size],
                                    start=(pr == 0),
                                    stop=False,
                                    perf_mode=DR,
                                )
                            # += CPOW*(z + B): b8*psum = z - conv(y)/L + B
                            nc.tensor.matmul(
                                pg[:, :size], I_C, zbsl, start=False, stop=True
                            )
                        else:
                            for d in range(ND):
                                nc.tensor.matmul(
                                    pg[:, :size],
                                    lhsT(1, d, q),
                                    YF[:, off + d : off + d + size],
                                    start=(d == 0),
                                    stop=False,
                                )
                            nc.tensor.matmul(
                                pg[:, :size], I_1, zbsl, start=False, stop=True
                            )
                        pgs.append(pg)
                    zsl = slice(BS + off, BS + off + size)
                    last_it = i == n_iter - 1
                    for q in range(2):
                        # scaled state: w' = Sz*w = relu(Sz*(..) - Sz*thr);
                        # Sz*thr == LMBD exactly.  Final iteration emits
                        # plain w for the bf16 output convt.
                        if i == 0:
                            nc.scalar.activation(
                                Wcur[q][:, zsl],
                                B[q][:, zsl],
                                mybir.ActivationFunctionType.Relu,
                                bias=TH if last_it else NLM,
                                scale=1.0 if last_it else SZ,
                            )
                        else:
                            if last_it:
                                sc_ = B8 if fp8 else 1.0
                                bi_ = TH
                            else:
                                sc_ = BS8 if fp8 else SZ
                                bi_ = NLM
                            nc.scalar.activation(
                                Wcur[q][:, zsl],
                                pgs[q][:, :size],
                                mybir.ActivationFunctionType.Relu,
                                bias=bi_,
                                scale=sc_,
                            )
                    if ti == len(z_tiles) - 1:
                        # zero the invalid (s >= Tz%8) lanes of the last block
                        for q in range(2):
                            nc.vector.tensor_scalar(
                                Wcur[q][:, lastz : lastz + 1],
                                Wcur[q][:, lastz : lastz + 1],
                                MK[:, q : q + 1],
                                None,
                                op0=AOP.mult,
                            )
                    if i < n_iter - 1:
                        # momentum + fp8-z in coarse chunks, emitted as soon
                        # as the covering tiles' w is final (keeps the next
                        # iteration's convt dependencies early in each
                        # engine's FIFO)
                        CH_COLS = 2 * ft
                        c0 = ti // 2 * CH_COLS if ti % 2 == 1 else None
                        if ti == len(z_tiles) - 1 and len(z_tiles) % 2 == 1:
                            c0 = (len(z_tiles) - 1) // 2 * CH_COLS
                        if c0 is None or c0 >= NZB:
                            continue
                        cs = min(CH_COLS, NZB - c0)
                        csl = slice(BS + c0, BS + c0 + cs)
                        for q in range(2):
                            # Wcur holds w' = Sz*w here (non-final iters)
                            if i == 0:
                                if next_fp8:
                                    nc.vector.tensor_copy(
                                        Z8[:, q, csl], Wcur[q][:, csl]
                                    )
                                else:
                                    nc.vector.tensor_scalar(
                                        Z[q][:, csl],
                                        Wcur[q][:, csl],
                                        ISZ,
                                        None,
                                        op0=AOP.mult,
                                    )
                                nc.vector.scalar_tensor_tensor(
                                    ZB[q][:, csl],
                                    Wcur[q][:, csl],
                                    ISZ,
                                    B[q][:, csl],
                                    AOP.mult,
                                    AOP.add,
                                )
                            else:
                                dt_ = tmps.tile(
                                    [128, CH_COLS], BF16, name="dt", tag="dt"
                                )
                                nc.vector.tensor_tensor(
                                    dt_[:, :cs],
                                    Wcur[q][:, csl],
                                    Wold[q][:, csl],
                                    op=AOP.subtract,
                                )
                                if next_fp8:
                                    # z8 = Sz*z directly: (dt' * m) + w'
                                    nc.vector.scalar_tensor_tensor(
                                        Z8[:, q, csl],
                                        dt_[:, :cs],
                                        mom,
                                        Wcur[q][:, csl],
                                        AOP.mult,
                                        AOP.add,
                                    )
                                else:
                                    # bf16 next iter: plain z = scaled/Sz
                                    nc.vector.scalar_tensor_tensor(
                                        Z[q][:, csl],
                                        dt_[:, :cs],
                                        mom,
                                        Wcur[q][:, csl],
                                        AOP.mult,
                                        AOP.add,
                                    )
                                    nc.vector.tensor_scalar(
                                        Z[q][:, csl],
                                        Z[q][:, csl],
                                        ISZ,
                                        None,
                                        op0=AOP.mult,
                                    )
                                wbc = tmps.tile(
                                    [128, CH_COLS], BF16, name="wbc", tag="wbc"
                                )
                                nc.vector.scalar_tensor_tensor(
                                    wbc[:, :cs],
                                    Wcur[q][:, csl],
                                    ISZ,
                                    B[q][:, csl],
                                    AOP.mult,
                                    AOP.add,
                                )
                                # ZB = z + B = (m/Sz)*dt' + (w'/Sz + B)
                                nc.vector.scalar_tensor_tensor(
                                    ZB[q][:, csl],
                                    dt_[:, :cs],
                                    SC[:, 7 + i : 8 + i],
                                    wbc[:, :cs],
                                    AOP.mult,
                                    AOP.add,
                                )

            # ---- iteration 0 (y = 0: w1 = relu(B - thr), no GEMMs) ----
            conv_update(0, WA, WB)
            cur, other = WA, WB

            # ---- iterations 1 .. n_iter-1 ----
            for i in range(1, n_iter):
                fp8 = i < n_fp8

                def consume_y(off, size, py, _fp8=fp8):
                    write_y(off, size, py, _fp8)

                if fp8:
                    convt_pass8(consume_y)
                else:
                    convt_pass(Z, consume_y)
                conv_update(i, other, cur)
                cur, other = other, cur

            # ---- output = convt(w_final), always bf16 ----
            def consume_out(off, size, py):
                nc.scalar.copy(OUT[:, off : off + size], py[:, :size])
                nc.sync.dma_start(out_d[:, off : off + size], OUT[:, off : off + size])

            convt_pass(cur, consume_out)

    nc.compile()
    return nc


_BUILD_CACHE: dict = {}


def _get_nc(T: int, n_iter: int, n_fp8: int = N_FP8):
    key = (T, n_iter, n_fp8)
    if key not in _BUILD_CACHE:
        _BUILD_CACHE[key] = build_nc(T, n_iter, n_fp8)
    return _BUILD_CACHE[key]


def host_inputs(D: np.ndarray, T: int, n_iter: int = N_ITER):
    """All data-dependent device inputs except x (shared across cores)."""
    import ml_dtypes

    L = _lipschitz(D)
    Sz = L
    wts = make_weights(D, L).astype(ml_dtypes.bfloat16)
    wts8 = make_weights8(D, L)
    moms = _momentum_schedule(n_iter)
    scal = np.zeros((128, 7 + n_iter), np.float32)
    scal[:, 0] = -(LMBD / L)
    scal[:, 1] = SY / (SW * Sz)
    scal[:, 2] = 1.0 / CPOW
    scal[:, 3] = Sz
    scal[:, 4] = Sz / CPOW
    scal[:, 5] = 1.0 / Sz
    scal[:, 6] = -LMBD
    for i in range(n_iter):
        scal[:, 7 + i] = moms[i] / Sz
    Tz = T - KS + 1
    NZB = -(-Tz // BS)
    vb = Tz - (NZB - 1) * BS
    masks = np.zeros((2, 128, 1), np.float32)
    for q in range(2):
        for p in range(128):
            masks[q, p, 0] = 1.0 if (4 * q + p // 32) < vb else 0.0
    return {"wts": wts, "wts8": wts8, "scal": scal, "masks": masks}


# test-harness hooks: set TRACE=True before calling kernel() to capture a
# neuron-profile; the measured exec time lands in LAST_RESULT.exec_time_ns.
TRACE = False
LAST_RESULT = None


def kernel(x: np.ndarray, D: np.ndarray) -> np.ndarray:
    import ml_dtypes

    from concourse.bass_utils import run_bass_kernel_spmd

    x = np.asarray(x, np.float32)
    D = np.asarray(D, np.float32)
    N, C, T = x.shape
    assert N == 8 and C == CH and D.shape == (KA, CH, KS)

    shared = host_inputs(D, T)
    nc = _get_nc(T, N_ITER, N_FP8)
    in_maps = [
        {"x": block_x(x[n]).astype(ml_dtypes.bfloat16), **shared} for n in range(N)
    ]
    res = run_bass_kernel_spmd(nc, in_maps, list(range(N)), trace=TRACE)
    global LAST_RESULT
    LAST_RESULT = res
    out = np.stack(
        [
            unblock_out(np.asarray(res.results[i]["out"], np.float32), T)
            for i in range(N)
        ],
        axis=0,
    )
    return out.astype(np.float32)


if __name__ == "__main__":
    nc = build_nc(32768, N_ITER)
    print("built OK")


# revision 27
# speedup vs baseline: 1.4995x; 1.0876x over previous
"""Trainium2 Bass kernel for convolutional sparse coding (FISTA, 20 iters).

Problem: x (8, 16, 32768) f32, D (32, 16, 64) f32.
    z = FISTA(x, D)  (20 iterations of grad = conv(convt(z, D) - x, D),
    soft-threshold, Nesterov momentum); output = convt(w_final, D).

Strategy (pure data parallel, one batch element per NeuronCore, D replicated):
  * Block-Toeplitz formulation with time blocks of BS=8: both convolutions
    become 9 GEMM "diagonals" of (128-contraction x 128-out x free) matmuls
    at full 128x128 PE utilization.
  * Layouts: z/w live as 2 SBUF tiles with partition (s%4)*32+k (s = in-block
    time, k = atom); y/x live as 1 tile with partition u*16+c.  The convt
    output layout IS the conv input layout, so no relayout ever happens and
    the whole 20-iteration FISTA state stays resident in SBUF.
  * y-split: grad = conv(convt(z))/L - B with B = conv(x)/L precomputed ONCE
    in bf16 and added into the gradient PSUM via a host-scaled identity
    matmul.  This removes the large exact x term from the per-iteration
    quantized path, so both per-iteration convolutions can run in fp8(e4m3)
    DoubleRow matmuls (2 MACs/cell/cycle, contraction pairs in the AP middle
    dim) with ~3e-3 final relative error.  Iterations >= N_FP8 fall back to
    bf16 matmuls (N_FP8=20 by default: all fp8).
  * Master z/w state is bf16; PSUM accumulates fp32; the soft-threshold
    reads PSUM at fp32.  All data-dependent scales travel as per-partition
    scalar vectors or host-built weight values, so the compiled NEFF depends
    only on shapes.
"""

import os
import sys

import numpy as np

for _p in (
    "/root/.axon_site",
    "/root/.axon_site/_ro/trn_rl_repo",
    "/root/.axon_site/_ro/pypackages",
    "/opt/trn_rl_repo",
):
    if os.path.isdir(_p) and _p not in sys.path:
        sys.path.append(_p)

import concourse.bass as bass  # noqa: E402
import concourse.tile as tile  # noqa: E402
from concourse import bacc, mybir  # noqa: E402

BS = 8  # time-block size
KS = 64  # conv kernel length
ND = KS // BS + 1  # 9 gemm diagonals
NPR = (ND + 1) // 2  # 5 fp8 DoubleRow diagonal-pairs
KA = 32  # dictionary atoms
CH = 16  # channels
LMBD = 0.1
N_ITER = 20
N_FP8 = 20  # iterations 0..N_FP8-1 use fp8 DoubleRow; rest bf16
FT = 512  # free-dim (block-index) tile size == one PSUM bank of fp32

SY = 32.0  # y is stored scaled by SY in fp8 mode
SW = 64.0  # fp8 convt weight pre-scale (|D| <= 1 so SW*|D| <= 64 << 448)
CPOW = float(2 ** 21)  # gradient-PSUM scale; exact in bf16 (identity value)

F32 = mybir.dt.float32
BF16 = mybir.dt.bfloat16
FP8 = mybir.dt.float8e4

# bf16 weight-set indices: [kind, d, q] with kinds
#   0: convt W (D), 1: conv V (-D/(SY*L)), 2: B-precompute VB (+D/L)
NW_BF = 3 * ND * 2 + 2  # + identity tiles: [54] = diag(CPOW), [55] = diag(1)
IDX_IC = 3 * ND * 2
IDX_I1 = 3 * ND * 2 + 1


def _lipschitz(D: np.ndarray) -> float:
    F = np.fft.fft(D.astype(np.float64), axis=2)
    L = float(np.sum(np.max((F * np.conj(F)).real, axis=2)))
    return 1.0 if L == 0.0 else L


def _momentum_schedule(n_iter: int) -> list[float]:
    betas = [1.0]
    for _ in range(n_iter):
        b = betas[-1]
        betas.append((1.0 + (1.0 + 4.0 * b * b) ** 0.5) / 2.0)
    return [(betas[i] - 1.0) / betas[i + 1] for i in range(n_iter)]


def _toeplitz_entry(D, d, u, s):
    j = BS * d + u - s
    if 0 <= j < KS:
        return D[:, :, j]  # [k, c]
    return None


def make_weights(D: np.ndarray, L: float) -> np.ndarray:
    """bf16 weights, shape (NW_BF, 128, 128), fp32 (cast later)."""
    D = np.asarray(D, np.float32)
    wts = np.zeros((NW_BF, 128, 128), np.float32)
    for kind, cs in ((0, 1.0), (1, -1.0 / (SY * L)), (2, 1.0 / L)):
        for d in range(ND):
            for q in range(2):
                M = np.zeros((4, KA, BS, CH), np.float32)  # (sl, k, u, c)
                V = np.zeros((BS, CH, 4, KA), np.float32)  # (u, c, sl, k)
                for sl in range(4):
                    s = 4 * q + sl
                    for u in range(BS):
                        e = _toeplitz_entry(D, d, u, s)
                        if e is not None:
                            M[sl, :, u, :] = e
                            V[u, :, sl, :] = cs * e.T
                wts[kind * 2 * ND + d * 2 + q] = (
                    M.reshape(128, 128) if kind == 0 else V.reshape(128, 128)
                )
    wts[IDX_IC] = np.eye(128, dtype=np.float32) * CPOW
    wts[IDX_I1] = np.eye(128, dtype=np.float32)
    return wts


def make_weights8(D: np.ndarray, L: float) -> np.ndarray:
    """fp8 DoubleRow weights, shape (ND + 2*NPR, 128, 2, 128), float8e4.

    convt tiles carry SW*D; conv tiles carry -(CPOW/(SY*L))*D so that the
    gradient PSUM is exactly CPOW*(z + B - conv(y)/L) with the z/B identity
    matmuls using the bf16-exact value CPOW."""
    import ml_dtypes

    SWC = CPOW / (SY * L)
    D = np.asarray(D, np.float32)
    w8 = np.zeros((ND + 2 * NPR, 128, 2, 128), np.float32)
    for d in range(ND):
        for j in range(2):
            M = np.zeros((4, KA, BS, CH), np.float32)
            for sl in range(4):
                for u in range(BS):
                    e = _toeplitz_entry(D, d, u, 4 * j + sl)
                    if e is not None:
                        M[sl, :, u, :] = SW * e
            w8[d, :, j, :] = M.reshape(128, 128)
    for pr in range(NPR):
        for q in range(2):
            for jj in range(2):
                d = 2 * pr + jj
                if d >= ND:
                    continue
                V = np.zeros((BS, CH, 4, KA), np.float32)
                for sl in range(4):
                    for u in range(BS):
                        e = _toeplitz_entry(D, d, u, 4 * q + sl)
                        if e is not None:
                            V[u, :, sl, :] = -SWC * e.T
                w8[ND + pr * 2 + q, :, jj, :] = V.reshape(128, 128)
    return w8.astype(ml_dtypes.float8_e4m3)


def block_x(xn: np.ndarray) -> np.ndarray:
    """(C, T) -> (128, T//BS) with partition u*16+c, col b = x[c, 8b+u]."""
    C, T = xn.shape
    return (
        np.ascontiguousarray(
            xn.reshape(C, T // BS, BS).transpose(2, 0, 1)
        ).reshape(128, T // BS)
    )


def unblock_out(ob: np.ndarray, T: int) -> np.ndarray:
    """(128, T//BS) -> (C, T)."""
    return np.ascontiguousarray(
        ob.reshape(BS, CH, T // BS).transpose(1, 2, 0)
    ).reshape(CH, T)


def build_nc(T: int, n_iter: int, n_fp8: int = N_FP8, ft: int = FT):
    """Build + compile the single-core FISTA graph (SPMD across 8 cores).

    DRAM params (host-prepped):
      x      (128, NB) bf16  pre-blocked x (B-precompute GEMM input)
      wts    (NW_BF, 128, 128) bf16
      wts8   (ND+2*NPR, 128, 2, 128) fp8e4
      scal   (128, 7+n_iter) f32  per-partition scalar columns:
             [-thr, ay, b8, Sz, Sz/CPOW, 1/Sz, -LMBD, m_i/Sz...]
      masks  (2, 128, 1) f32
      out    (128, NB) f32
    with Sz = L, ay = SY/(SW*Sz), b8 = 1/CPOW.
    """
    assert T % BS == 0
    NB = T // BS
    Tz = T - KS + 1
    NZB = -(-Tz // BS)
    ZC = NB + BS  # 8 left zero-pad + NZB data + right zero-pad
    YC = NB + 2  # one zero halo block (+1 spare: keep bytes %4)
    YC8 = -(-YC // 16) * 16  # fp8 plane tile: middle-dim step %16 bytes
    y_tiles = [(i * ft, min(ft, NB - i * ft)) for i in range(-(-NB // ft))]
    z_tiles = [(i * ft, min(ft, NZB - i * ft)) for i in range(-(-NZB // ft))]
    moms = _momentum_schedule(n_iter)
    lastz = BS + NZB - 1  # column of the (partially valid) last z block
    DR = mybir.MatmulPerfMode.DoubleRow
    AOP = mybir.AluOpType

    nc = bacc.Bacc("TRN2", target_bir_lowering=False, debug=False)
    x_d = nc.declare_dram_parameter("x", [128, NB], BF16, isOutput=False)
    wts_d = nc.declare_dram_parameter("wts", [NW_BF, 128, 128], BF16, isOutput=False)
    wts8_d = nc.declare_dram_parameter(
        "wts8", [ND + 2 * NPR, 128, 2, 128], FP8, isOutput=False
    )
    scal_d = nc.declare_dram_parameter(
        "scal", [128, 7 + n_iter], F32, isOutput=False
    )
    mask_d = nc.declare_dram_parameter("masks", [2, 128, 1], F32, isOutput=False)
    out_d = nc.declare_dram_parameter("out", [128, NB], F32, isOutput=True)

    with tile.TileContext(nc) as tc:
        with (
            tc.tile_pool(name="state", bufs=1) as state,
            tc.tile_pool(name="py", bufs=2, space="PSUM") as py_pool,
            tc.tile_pool(name="pg", bufs=6, space="PSUM") as pg_pool,
            tc.tile_pool(name="tmps", bufs=6) as tmps,
        ):
            XB = state.tile([128, YC], BF16, name="XB")
            YF = state.tile([128, YC], BF16, name="YF")
            Y2 = state.tile([128, 2, YC8], FP8, name="Y2")
            Z = [state.tile([128, ZC], BF16, name=f"Zq{q}") for q in range(2)]
            Z8 = state.tile([128, 2, -(-ZC // 16) * 16], FP8, name="Z8")
            B = [state.tile([128, ZC], BF16, name=f"Bq{q}") for q in range(2)]
            WA = [state.tile([128, ZC], BF16, name=f"WAq{q}") for q in range(2)]
            WB = [state.tile([128, ZC], BF16, name=f"WBq{q}") for q in range(2)]
            WT = state.tile([128, NW_BF * 128], BF16, name="WT")
            WT8 = state.tile([128, ND + 2 * NPR, 2, 128], FP8, name="WT8")
            SC = state.tile([128, 7 + n_iter], F32, name="SC")
            MK = state.tile([128, 2], F32, name="MK")
            OUT = state.tile([128, NB], F32, name="OUTS")

            nc.sync.dma_start(XB[:, :NB], x_d[:])
            nc.sync.dma_start(
                WT[:].rearrange("p (i j) -> p i j", i=NW_BF),
                wts_d[:].rearrange("i p j -> p i j"),
            )
            nc.sync.dma_start(WT8[:], wts8_d[:].rearrange("i p j c -> p i j c"))
            nc.sync.dma_start(SC[:], scal_d[:])
            nc.sync.dma_start(
                MK[:].rearrange("p (q o) -> p q o", q=2),
                mask_d[:].rearrange("q p o -> p q o"),
            )
            TH = SC[:, 0:1]  # -thr
            AY = SC[:, 1:2]  # SY/(SW*Sz)
            B8 = SC[:, 2:3]  # 1/CPOW
            SZ = SC[:, 3:4]  # Sz
            BS8 = SC[:, 4:5]  # Sz/CPOW (relu scale for scaled-w state)
            ISZ = SC[:, 5:6]  # 1/Sz
            NLM = SC[:, 6:7]  # -LMBD (= -Sz*thr, bias in scaled-w space)

            # zero pads / halos once via f32-bitcast full-tile memsets
            nc.vector.memset(XB[:, NB:YC].bitcast(F32), 0.0)
            for q in range(2):
                for W2 in (Z, WA, WB):
                    nc.vector.memset(W2[q][:, :].bitcast(F32), 0.0)
            nc.vector.memset(YF[:, :].bitcast(F32), 0.0)
            nc.vector.memset(Y2[:, :, :].bitcast(F32), 0.0)
            nc.vector.memset(Z8[:, :, :].bitcast(F32), 0.0)

            def lhsT(kind: int, d: int, q: int):
                i = kind * 2 * ND + d * 2 + q
                return WT[:, i * 128 : (i + 1) * 128]

            I_C = WT[:, IDX_IC * 128 : (IDX_IC + 1) * 128]
            I_1 = WT[:, IDX_I1 * 128 : (IDX_I1 + 1) * 128]

            # ---- B = conv(x)/L in z-layout (one-time, bf16) ----
            for off, size in z_tiles:
                for q in range(2):
                    pb = pg_pool.tile([128, ft], F32, name="pbt", tag="pgt")
                    for d in range(ND):
                        nc.tensor.matmul(
                            pb[:, :size],
                            lhsT(2, d, q),
                            XB[:, off + d : off + d + size],
                            start=(d == 0),
                            stop=(d == ND - 1),
                        )
                    nc.scalar.copy(B[q][:, BS + off : BS + off + size], pb[:, :size])

            def convt_pass(SRC, consume):
                """bf16: psum_y[ft] = sum_d Wd^T SRC_blocks (= y)."""
                for off, size in y_tiles:
                    py = py_pool.tile([128, ft], F32, name="pyt", tag="pyt")
                    k = 0
                    for d in range(ND):
                        for q in range(2):
                            lo = BS + off - d
                            nc.tensor.matmul(
                                py[:, :size],
                                lhsT(0, d, q),
                                SRC[q][:, lo : lo + size],
                                start=(k == 0),
                                stop=(k == 2 * ND - 1),
                            )
                            k += 1
                    consume(off, size, py)

            def convt_pass8(consume):
                """fp8 DR: psum_y[ft] = SW*Sz*y from Z8."""
                for off, size in y_tiles:
                    py = py_pool.tile([128, ft], F32, name="pyt", tag="pyt")
                    for d in range(ND):
                        lo = BS + off - d
                        nc.tensor.matmul(
                            py[:, :size],
                            WT8[:, d, :, :],
                            Z8[:, :, lo : lo + size],
                            start=(d == 0),
                            stop=(d == ND - 1),
                            perf_mode=DR,
                        )
                    consume(off, size, py)

            def write_y(off, size, py, fp8: bool):
                """Y2 planes (= SY*y fp8) or YF (= SY*y bf16) from psum."""
                if fp8:
                    # psum = SW*Sz*y -> y8 = ay*psum  (on ACT)
                    nc.scalar.activation(
                        Y2[:, 0, off : off + size],
                        py[:, :size],
                        mybir.ActivationFunctionType.Copy,
                        bias=0.0,
                        scale=AY,
                    )
                    # plane1 = plane0 shifted left by one block; plane1[NB-1]
                    # stays 0 from the initial memset (= zero halo)
                    dlo = max(off - 1, 0)
                    nc.vector.tensor_copy(
                        Y2[:, 1, dlo : off + size - 1],
                        Y2[:, 0, dlo + 1 : off + size],
                    )
                else:
                    # psum = y -> yf = SY*psum
                    nc.vector.tensor_scalar(
                        YF[:, off : off + size],
                        py[:, :size],
                        float(SY),
                        None,
                        op0=AOP.mult,
                    )

            def conv_update(i: int, Wcur, Wold):
                """gradient GEMMs + B + soft-threshold + momentum, iter i."""
                fp8 = i < n_fp8
                next_fp8 = (i + 1) < n_fp8
                mom = float(moms[i])
                for ti, (off, size) in enumerate(z_tiles):
                    pgs = []
                    for q in range(2):
                        if i == 0:
                            pgs.append(None)  # t2 comes straight from B
                            continue
                        pg = pg_pool.tile([128, ft], F32, name=f"pgt{q}", tag="pgt")
                        bsl = B[q][:, BS + off : BS + off + size]
                        zslq = Z[q][:, BS + off : BS + off + size]
                        if fp8:
                            for pr in range(NPR):
                                nc.tensor.matmul(
                                    pg[:, :size],
                                    WT8[:, ND + pr * 2 + q, :, :],
                                    Y2[:, :, off + 2 * pr : off + 2 * pr + size],
                                    start=(pr == 0),
                                    stop=False,
                                    perf_mode=DR,
                                )
                            # += CPOW*(B + z): b8*psum = z - conv(y)/L + B
                            nc.tensor.matmul(
                                pg[:, :size], I_C, bsl, start=False, stop=False
                            )
                            nc.tensor.matmul(
                                pg[:, :size], I_C, zslq, start=False, stop=True
                            )
                        else:
                            for d in range(ND):
                                nc.tensor.matmul(
                                    pg[:, :size],
                                    lhsT(1, d, q),
                                    YF[:, off + d : off + d + size],
                                    start=(d == 0),
                                    stop=False,
                                )
                            nc.tensor.matmul(
                                pg[:, :size], I_1, bsl, start=False, stop=False
                            )
                            nc.tensor.matmul(
                                pg[:, :size], I_1, zslq, start=False, stop=True
                            )
                        pgs.append(pg)
                    zsl = slice(BS + off, BS + off + size)
                    for q in range(2):
                        if i == 0:
                            # z == 0, y == 0: w = relu(B - thr)
                            nc.scalar.activation(
                                Wcur[q][:, zsl],
                                B[q][:, zsl],
                                mybir.ActivationFunctionType.Relu,
                                bias=TH,
                                scale=1.0,
                            )
                        else:
                            nc.scalar.activation(
                                Wcur[q][:, zsl],
                                pgs[q][:, :size],
                                mybir.ActivationFunctionType.Relu,
                                bias=TH,
                                scale=B8 if fp8 else 1.0,
                            )
                    if ti == len(z_tiles) - 1:
                        # zero the invalid (s >= Tz%8) lanes of the last block
                        for q in range(2):
                            nc.vector.tensor_scalar(
                                Wcur[q][:, lastz : lastz + 1],
                                Wcur[q][:, lastz : lastz + 1],
                                MK[:, q : q + 1],
                                None,
                                op0=AOP.mult,
                            )
                    if i < n_iter - 1:
                        # momentum + fp8-z in coarse chunks, emitted as soon
                        # as the covering tiles' w is final (keeps the next
                        # iteration's convt dependencies early in each
                        # engine's FIFO)
                        CH_COLS = 2 * ft
                        c0 = ti // 2 * CH_COLS if ti % 2 == 1 else None
                        if ti == len(z_tiles) - 1 and len(z_tiles) % 2 == 1:
                            c0 = (len(z_tiles) - 1) // 2 * CH_COLS
                        if c0 is None or c0 >= NZB:
                            continue
                        cs = min(CH_COLS, NZB - c0)
                        csl = slice(BS + c0, BS + c0 + cs)
                        for q in range(2):
                            if i == 0:
                                nc.vector.tensor_copy(Z[q][:, csl], Wcur[q][:, csl])
                            else:
                                dt_ = tmps.tile(
                                    [128, CH_COLS], BF16, name="dt", tag="dt"
                                )
                                nc.vector.tensor_tensor(
                                    dt_[:, :cs],
                                    Wcur[q][:, csl],
                                    Wold[q][:, csl],
                                    op=AOP.subtract,
                                )
                                nc.vector.scalar_tensor_tensor(
                                    Z[q][:, csl],
                                    dt_[:, :cs],
                                    mom,
                                    Wcur[q][:, csl],
                                    AOP.mult,
                                    AOP.add,
                                )
                            if next_fp8:
                                # fp8 copy of z for next iter (ACT): Sz * Z
                                nc.scalar.activation(
                                    Z8[:, q, csl],
                                    Z[q][:, csl],
                                    mybir.ActivationFunctionType.Copy,
                                    bias=0.0,
                                    scale=SZ,
                                )

            # ---- iteration 0 (y = 0: w1 = relu(B - thr), no GEMMs) ----
            conv_update(0, WA, WB)
            cur, other = WA, WB

            # ---- iterations 1 .. n_iter-1 ----
            for i in range(1, n_iter):
                fp8 = i < n_fp8

                def consume_y(off, size, py, _fp8=fp8):
                    write_y(off, size, py, _fp8)

                if fp8:
                    convt_pass8(consume_y)
                else:
                    convt_pass(Z, consume_y)
                conv_update(i, other, cur)
                cur, other = other, cur

            # ---- output = convt(w_final), always bf16 ----
            def consume_out(off, size, py):
                nc.scalar.copy(OUT[:, off : off + size], py[:, :size])
                nc.sync.dma_start(out_d[:, off : off + size], OUT[:, off : off + size])

            convt_pass(cur, consume_out)

    nc.compile()
    return nc


_BUILD_CACHE: dict = {}


def _get_nc(T: int, n_iter: int, n_fp8: int = N_FP8):
    key = (T, n_iter, n_fp8)
    if key not in _BUILD_CACHE:
        _BUILD_CACHE[key] = build_nc(T, n_iter, n_fp8)
    return _BUILD_CACHE[key]


def host_inputs(D: np.ndarray, T: int, n_iter: int = N_ITER):
    """All data-dependent device inputs except x (shared across cores)."""
    import ml_dtypes

    L = _lipschitz(D)
    Sz = L
    wts = make_weights(D, L).astype(ml_dtypes.bfloat16)
    wts8 = make_weights8(D, L)
    moms = _momentum_schedule(n_iter)
    scal = np.zeros((128, 7 + n_iter), np.float32)
    scal[:, 0] = -(LMBD / L)
    scal[:, 1] = SY / (SW * Sz)
    scal[:, 2] = 1.0 / CPOW
    scal[:, 3] = Sz
    scal[:, 4] = Sz / CPOW
    scal[:, 5] = 1.0 / Sz
    scal[:, 6] = -LMBD
    for i in range(n_iter):
        scal[:, 7 + i] = moms[i] / Sz
    Tz = T - KS + 1
    NZB = -(-Tz // BS)
    vb = Tz - (NZB - 1) * BS
    masks = np.zeros((2, 128, 1), np.float32)
    for q in range(2):
        for p in range(128):
            masks[q, p, 0] = 1.0 if (4 * q + p // 32) < vb else 0.0
    return {"wts": wts, "wts8": wts8, "scal": scal, "masks": masks}


# test-harness hooks: set TRACE=True before calling kernel() to capture a
# neuron-profile; the measured exec time lands in LAST_RESULT.exec_time_ns.
TRACE = False
LAST_RESULT = None


def kernel(x: np.ndarray, D: np.ndarray) -> np.ndarray:
    import ml_dtypes

    from concourse.bass_utils import run_bass_kernel_spmd

    x = np.asarray(x, np.float32)
    D = np.asarray(D, np.float32)
    N, C, T = x.shape
    assert N == 8 and C == CH and D.shape == (KA, CH, KS)

    shared = host_inputs(D, T)
    nc = _get_nc(T, N_ITER, N_FP8)
    in_maps = [
        {"x": block_x(x[n]).astype(ml_dtypes.bfloat16), **shared} for n in range(N)
    ]
    res = run_bass_kernel_spmd(nc, in_maps, list(range(N)), trace=TRACE)
    global LAST_RESULT
    LAST_RESULT = res
    out = np.stack(
        [
            unblock_out(np.asarray(res.results[i]["out"], np.float32), T)
            for i in range(N)
        ],
        axis=0,
    )
    return out.astype(np.float32)


if __name__ == "__main__":
    nc = build_nc(32768, N_ITER)
    print("built OK")
